# revision 25
# baseline (speedup 1.0000x reference)
"""CrossAttention (cosine-sim, learnable temperature) Trainium2 kernel, v4.

Math (per batch element b, reference in fp32):
    qh  = (q @ Wq.T)   -> [Lq, C] -> heads [H, Lq, D]
    k,v = (kv @ Wkv.T) -> k,v [H, Lkv, D]
    qn = qh / ||qh||_d; kn = k / ||k||_d
    attn = softmax(qn @ kn.T / tau); out = attn @ v
    y = out @ Wproj.T + bproj         (bproj added on host)

Distribution: pure data-parallel over B=8 across the 8 NeuronCores (one
batch element per core, weights replicated, no collectives).

v4 design notes (changes vs v2 baseline, driven by NTFF profiles):
  * Both heads of a pair share ONE 4-bank PSUM scores tile and ONE
    bf16 pt tile, so the softmax Exp runs at N=2048 instead of 2x
    N=1024 - 64 ACTIVATEs instead of 128, amortizing the ~480ns
    per-instruction overhead.  Measured: 15.7us of Exp per pair vs
    21.4us in v2.  The scores tile is single-buffered (4 banks); the
    interleaved PV/filler matmuls cover the Exp latency between kt
    steps.
  * Dummy zero-scale Sqrt/Exp activations preload the ACT table sets
    (sqrt set at kernel start, exp set during the phase boundary), so
    the first real Exp doesn't stall scores - in the v2 trace that
    stall tipped the PE into a 37us HAM re-throttle covering pairs
    0-1.
  * The norm chain and the softmax-sum normalization use PE
    ones-matmul broadcasts exactly as v2 (a v3 experiment with
    SBUF->SBUF broadcast DMAs was 2x slower: the DMA path runs at
    ~58GB/s and its queue backs up ~20us).
  * kv-ch0 input DMAs are issued before the wk columns (the first K
    job needs all kv chunks but only one wk column), cutting the DMA
    lead-in before the first matmul.
  * The phase-2 V-proj kv blocks and wv second half are DMA'd during
    phase 1 (their pools are allocated up front), so pair 0's filler
    never waits on DMA at the phase boundary.
  * O-proj filler spread over 3 partial stages (ct 0-2 at pairs 2-4,
    2-4 at 5-6, 4-6 during pair 7); the ct 6-8 finals interleave into
    pair 7's PV steps via a dedicated PSUM pool carved from the freed
    scores banks, shrinking the ACT-idle tail.
"""

import sys

sys.path.insert(0, "/opt/trn_rl_repo")

import numpy as np
import ml_dtypes

import concourse.bass as bass
import concourse.bacc as bacc
import concourse.mybir as mybir
from concourse.tile import TileContext
from concourse.bass_utils import run_bass_kernel_spmd

AF = mybir.ActivationFunctionType
F32 = mybir.dt.float32
F32R = mybir.dt.float32r
F16 = mybir.dt.float16
BF16 = mybir.dt.bfloat16

NCORES = 8


DEFAULT_KNOBS = dict(
    psA_bufs=4, psS_bufs=2, psB_bufs=2,
    sq_bufs=3, smalls_bufs=4, rbs_bufs=2,
    psSc_bufs=1, psPV_bufs=3, psBc_bufs=1,
    pt_bufs=2, rsum_bufs=2, sbb_bufs=3, tmp_bufs=2, y_bufs=2,
)


def build_nc(C=1024, H=16, LQ=1024, LKV=1024, knobs=None):
    kb = dict(DEFAULT_KNOBS)
    if knobs:
        kb.update(knobs)
    P = 128
    D = C // H            # head dim (64)
    OT = C // P           # feature tiles (8)
    CT = C // P           # contraction tiles (8)
    KT = LKV // P         # lkv partition tiles (8)
    HPT = P // D          # heads per 128-tile (2)
    CH = min(512, LQ)     # free-dim chunk per psum bank (fp32)
    NCH = LQ // CH        # chunks of Lq (2)
    VCH = min(512, C)     # chunk of output features for V projection
    NVCH = C // VCH
    HPC = VCH // D        # heads per v-projection chunk (8)

    nc = bacc.Bacc("TRN2", target_bir_lowering=False)

    qT = nc.dram_tensor("qT", [C, LQ], F16, kind="ExternalInput")
    kvT = nc.dram_tensor("kvT", [C, LKV], F16, kind="ExternalInput")
    wqT = nc.dram_tensor("wqT", [C, C], F16, kind="ExternalInput")
    wkT = nc.dram_tensor("wkT", [C, C], F16, kind="ExternalInput")
    wvT = nc.dram_tensor("wvT", [C, C], F16, kind="ExternalInput")
    wpT = nc.dram_tensor("wpT", [C, C], BF16, kind="ExternalInput")
    tau2 = nc.dram_tensor("tau2", [HPT, 1], F32, kind="ExternalInput")
    ones_blk = nc.dram_tensor("ones_blk", [P, HPT], F16, kind="ExternalInput")
    blk2 = nc.dram_tensor("blk2", [HPT, P], F16, kind="ExternalInput")
    y = nc.dram_tensor("y", [LQ, C], F32, kind="ExternalOutput")

    qT_r = qT.rearrange("(ct p) l -> p ct l", p=P)
    kvT_r = kvT.rearrange("(ct p) l -> p ct l", p=P)
    wqT_r = wqT.rearrange("(ct p) o -> p ct o", p=P)
    wkT_r = wkT.rearrange("(ct p) o -> p ct o", p=P)
    wvT_r = wvT.rearrange("(ct p) o -> p ct o", p=P)
    wpT_r = wpT.rearrange("(ct p) o -> p ct o", p=P)
    y_r = y.rearrange("(yt p) o -> p yt o", p=P)

    with TileContext(nc) as tc:
        from contextlib import ExitStack

        with ExitStack() as stk:
            # ---------- persistent pools --------------------------------
            persist = stk.enter_context(tc.tile_pool(name="persist", bufs=1))
            qnT = persist.tile([P, OT, LQ], F16)            # qh * rq
            knT = persist.tile([P, OT, LKV], F16)           # kh * rk / tau
            v_aug = persist.tile([P, KT, H, D + 1], BF16)   # [v | ones]
            oT = persist.tile([P, CT, LQ], BF16)            # (attn@v)/sum
            wp_sb = persist.tile([P, CT, C], BF16)
            consts = stk.enter_context(tc.tile_pool(name="consts", bufs=1))
            ones_blk_sb = consts.tile([P, HPT], F16)
            blk2_sb = consts.tile([HPT, P], F16)
            tau2_sb = consts.tile([HPT, 1], F32)
            ones64 = consts.tile([1, D], BF16)
            scr = consts.tile([HPT, 1], F32)

            nc.sync.dma_start(out=ones_blk_sb, in_=ones_blk[:, :])
            nc.sync.dma_start(out=blk2_sb, in_=blk2[:, :])
            nc.sync.dma_start(out=tau2_sb, in_=tau2[:, :])
            nc.vector.memset(ones64, 1.0)
            nc.vector.memset(v_aug[:, :, :, D : D + 1], 1.0)
            # Preload the sqrt ACT table set before the first real Sqrt.
            nc.scalar.activation(scr, tau2_sb, AF.Sqrt, scale=0.0)

            # Phase-2 V-proj inputs: pools allocated up front (they must
            # outlive phase 1's pools on the stack) and DMA'd during
            # phase 1, so pair 0's filler never waits at the boundary.
            wv1p = stk.enter_context(tc.tile_pool(name="wv1p", bufs=1))
            wv1_sb = wv1p.tile([P, CT, VCH], F16)
            kvbp = stk.enter_context(tc.tile_pool(name="kvbp", bufs=KT))
            kvb_tiles = []
            for vt in range(KT):
                kvb = kvbp.tile([P, CT, P], F16, tag="kvb", name="kvb")
                kvb_tiles.append(kvb)

            # ---------- phase 1 (scoped so pools free before phase 2) ----
            p1 = ExitStack()
            # kv ch0 chunks first (the first K job needs all of them but
            # only one wk column), then wk/wv columns, kv ch1, then q + q
            # weights (phase 1b), O-proj weights and phase-2 V inputs last.
            # Inputs are split into per-chunk TILES (not slices of one big
            # tile): the Tile dependency tracker is tile-granular across
            # mismatched AP patterns, so one big tile makes the first matmul
            # wait for ALL input DMAs (~19us) instead of just the chunks it
            # reads (~8us).
            p1w = p1.enter_context(tc.tile_pool(name="p1w", bufs=1))
            kv_tiles = [[p1w.tile([P, CH], F16, name=f"kv_{ct}_{ch}")
                         for ch in range(NCH)] for ct in range(CT)]
            wk_tiles = [p1w.tile([P, CT, P], F16, name=f"wk_{ot}")
                        for ot in range(OT)]
            wv_sb = p1w.tile([P, CT, VCH], F16)   # first half only (vch 0)
            q_tiles = [[p1w.tile([P, CH], F16, name=f"q_{ct}_{ch}")
                        for ch in range(NCH)] for ct in range(CT)]
            wq_tiles = [p1w.tile([P, CT, P], F16, name=f"wq_{ot}")
                        for ot in range(OT)]
            for ct in range(CT):
                nc.sync.dma_start(out=kv_tiles[ct][0], in_=kvT_r[:, ct, 0:CH])
            for ot in range(3):
                sl = slice(ot * P, (ot + 1) * P)
                nc.sync.dma_start(out=wk_tiles[ot], in_=wkT_r[:, :, sl])
            for ct in range(CT):
                nc.sync.dma_start(out=kv_tiles[ct][1], in_=kvT_r[:, ct, CH:LKV])
            for ot in range(3, OT):
                sl = slice(ot * P, (ot + 1) * P)
                nc.sync.dma_start(out=wk_tiles[ot], in_=wkT_r[:, :, sl])
            for ct in range(VCH // P):
                sl = slice(ct * P, (ct + 1) * P)
                nc.sync.dma_start(out=wv_sb[:, :, sl], in_=wvT_r[:, :, sl])
            for ct in range(CT):
                nc.sync.dma_start(out=q_tiles[ct][0], in_=qT_r[:, ct, 0:CH])
                nc.sync.dma_start(out=q_tiles[ct][1], in_=qT_r[:, ct, CH:LQ])
            for ot in range(OT):
                sl = slice(ot * P, (ot + 1) * P)
                nc.sync.dma_start(out=wq_tiles[ot], in_=wqT_r[:, :, sl])
            for ct in range(CT):
                nc.sync.dma_start(out=wp_sb[:, ct, :], in_=wpT_r[:, ct, :])
            for ct in range(CT):
                nc.sync.dma_start(
                    out=wv1_sb[:, ct, :], in_=wvT_r[:, ct, VCH : 2 * VCH]
                )
            for vt in range(KT):
                nc.sync.dma_start(
                    out=kvb_tiles[vt], in_=kvT_r[:, :, vt * P : (vt + 1) * P]
                )

            # ============ PHASE 1a: K norm-proj + V proj ================
            class Job:
                def A(self):
                    pass

                def B(self):
                    pass

                def Cs(self):
                    pass

            def run_pipeline(jobs):
                n = len(jobs)
                for i in range(n + 2):
                    if i < n:
                        jobs[i].A()
                    if 0 <= i - 1 < n:
                        jobs[i - 1].B()
                    if 0 <= i - 2 < n:
                        jobs[i - 2].Cs()

            with ExitStack() as p1c:
                sqp = p1c.enter_context(tc.tile_pool(name="sqp", bufs=kb["sq_bufs"]))
                smalls = p1c.enter_context(
                    tc.tile_pool(name="smalls", bufs=kb["smalls_bufs"])
                )
                rbs = p1c.enter_context(tc.tile_pool(name="rbsa", bufs=kb["rbs_bufs"]))
                psA = p1c.enter_context(
                    tc.tile_pool(name="psA", bufs=kb["psA_bufs"], space="PSUM")
                )
                psS = p1c.enter_context(
                    tc.tile_pool(name="psS", bufs=kb["psS_bufs"], space="PSUM")
                )
                psB = p1c.enter_context(
                    tc.tile_pool(name="psB", bufs=kb["psB_bufs"], space="PSUM")
                )

                class NormJob(Job):
                    """Shared K/Q norm-projection job body (v2 chain)."""

                    def __init__(self, ot, ch):
                        self.ot, self.ch = ot, ch
                        self.sl = slice(ch * CH, (ch + 1) * CH)

                    def A(self):
                        self.ph = psA.tile([P, CH], F32, tag="ph", name="ph")
                        w_tiles, x_tiles = self.srcs()
                        wcol = w_tiles[self.ot]
                        for ct in range(CT):
                            nc.tensor.matmul(
                                self.ph,
                                wcol[:, ct, :],
                                x_tiles[ct][self.ch],
                                start=(ct == 0),
                                stop=(ct == CT - 1),
                            )
                        self.sq = sqp.tile([P, CH], F16, tag="sq", name="sq")
                        nc.scalar.activation(self.sq, self.ph, AF.Square)

                    def B(self):
                        ssq = psS.tile([HPT, CH], F32, tag="ssq", name="ssq")
                        nc.tensor.matmul(ssq, ones_blk_sb, self.sq, start=True, stop=True)
                        # rr = sqrt(ssq * scale) in f16, so the broadcast
                        # matmul below runs at full f16 rate
                        self.rr = smalls.tile([HPT, CH], F16, tag="rr", name="rr")
                        nc.scalar.activation(
                            self.rr, ssq, AF.Sqrt, scale=self.sqrt_scale()
                        )

                    def Cs(self):
                        rb = psB.tile([P, CH], F32, tag="rb", name="rb")
                        nc.tensor.matmul(rb, blk2_sb, self.rr, start=True, stop=True)
                        rb_sb = rbs.tile([P, CH], F32, tag="rb_sb", name="rb_sb")
                        nc.vector.reciprocal_approx_fast(rb_sb, rb)
                        nc.vector.tensor_mul(
                            self.dst()[:, self.ot, self.sl], self.ph, rb_sb
                        )

                class KJob(NormJob):
                    def srcs(self):
                        return wk_tiles, kv_tiles

                    def sqrt_scale(self):
                        return tau2_sb

                    def dst(self):
                        return knT

                class QJob(NormJob):
                    def srcs(self):
                        return wq_tiles, q_tiles

                    def sqrt_scale(self):
                        return 1.0

                    def dst(self):
                        return qnT

                class VJob(Job):
                    def __init__(self, vch, vt):
                        self.vch, self.vt = vch, vt

                    def A(self):
                        self.pv = psA.tile([P, VCH], F32, tag="ph", name="pv")
                        wcol = wv_sb[:, :, self.vch * VCH : (self.vch + 1) * VCH]
                        vch_blk, off = divmod(self.vt * P, CH)
                        for ct in range(CT):
                            nc.tensor.matmul(
                                self.pv,
                                kv_tiles[ct][vch_blk][:, off : off + P],
                                wcol[:, ct, :],
                                start=(ct == 0),
                                stop=(ct == CT - 1),
                            )

                    def Cs(self):
                        nc.vector.tensor_copy(
                            v_aug[
                                :, self.vt, self.vch * HPC : (self.vch + 1) * HPC, 0:D
                            ],
                            self.pv.rearrange("p (h d) -> p h d", d=D),
                        )

                # One merged pipeline: ch0 K jobs first (they only need the
                # ch0 kv halves), V jobs slotted in as their inputs land,
                # then ch1 K jobs, then all Q jobs.  A single pool scope
                # means no pipeline drain at the K/Q boundary.
                jobs = [KJob(ot, 0) for ot in range(4)]
                for i in range(4):
                    jobs += [KJob(4 + i, 0), VJob(0, i)]
                for i in range(4):
                    jobs += [KJob(i, 1), VJob(0, 4 + i)]
                jobs += [KJob(4 + i, 1) for i in range(4)]
                jobs += [QJob(i // 2, i % 2) for i in range(2 * OT)]
                run_pipeline(jobs)

            # Preload the ACT exp table set during the phase boundary so the
            # first real Exp doesn't stall the (single-buffered) scores tile.
            nc.scalar.activation(scr, tau2_sb, AF.Exp, scale=0.0)

            # free phase-1 inputs/weights before the big pt pool allocates
            p1.close()

            # ============ PHASE 2: attention (head pairs) ===============
            with ExitStack() as p2:
                ymp = p2.enter_context(tc.tile_pool(name="ymp", bufs=1))
                y_mid = ymp.tile([P, LQ // P, C], BF16)
                ptp = p2.enter_context(tc.tile_pool(name="ptp", bufs=kb["pt_bufs"]))
                rsp = p2.enter_context(tc.tile_pool(name="rsp", bufs=kb["rsum_bufs"]))
                sbb = p2.enter_context(tc.tile_pool(name="sbb", bufs=kb["sbb_bufs"]))
                tmpp = p2.enter_context(tc.tile_pool(name="tmpp", bufs=kb["tmp_bufs"]))
                yp = p2.enter_context(tc.tile_pool(name="yp", bufs=kb["y_bufs"]))
                psPV = p2.enter_context(
                    tc.tile_pool(name="psPV", bufs=kb["psPV_bufs"], space="PSUM")
                )
                psBc = p2.enter_context(
                    tc.tile_pool(name="psBc", bufs=kb["psBc_bufs"], space="PSUM")
                )
                # psSc entered last so it can be released (LIFO) before the
                # tail, freeing its 4 banks for the psO2 pool.
                psSc_ctx = ExitStack()
                psSc = psSc_ctx.enter_context(
                    tc.tile_pool(name="psSc", bufs=kb["psSc_bufs"], space="PSUM")
                )

                def emit_scores_step(ot, kt, pt01):
                    """One kt slice of a head pair's scores + exp: both heads
                    share a 4-bank PSUM tile so the Exp runs at N=2048.
                    Matmuls are ch-major so the hp0/hp1 matmuls sit on PE
                    row groups 0-1 / 2-3 and execute concurrently - the
                    scores leg of the serial scores->Exp chain is ~2 matmul
                    times, not 4."""
                    kl = slice(kt * P, (kt + 1) * P)
                    s01 = psSc.tile([P, HPT, LQ], F32, tag="ps_s", name="s01")
                    for ch in range(NCH):
                        sl = slice(ch * CH, (ch + 1) * CH)
                        for hp in range(HPT):
                            r = slice(hp * D, (hp + 1) * D)
                            nc.tensor.matmul(
                                s01[:, hp, sl], knT[r, ot, kl], qnT[r, ot, sl],
                                start=True, stop=True,
                            )
                    nc.scalar.activation(pt01[:, kt, :, :], s01, AF.Exp)

                def pv_mms(pair, hp, ch, pt01):
                    """attn@v (+softmax sum via the ones column) matmuls for
                    one (head, Lq-chunk)."""
                    sl = slice(ch * CH, (ch + 1) * CH)
                    pv = psPV.tile([D + 1, CH], F32, tag="ps_pv", name="ps_pv")
                    for kt in range(KT):
                        nc.tensor.matmul(
                            pv,
                            v_aug[:, kt, pair[0] + hp, :],
                            pt01[:, kt, hp, sl],
                            start=(kt == 0),
                            stop=(kt == KT - 1),
                        )
                    return pv

                def pv_tail(h, ch, pv):
                    """Softmax-sum fast-recip after a PE ones-broadcast,
                    then normalize into oT (v2 chain)."""
                    par, ot = h % HPT, h // HPT
                    sl = slice(ch * CH, (ch + 1) * CH)
                    sums = rsp.tile([1, CH], BF16, tag="rsum", name="sums")
                    nc.vector.tensor_copy(sums, pv[D : D + 1, :])
                    ps_b = psBc.tile([D, CH], F32, tag="ps_b", name="ps_b")
                    nc.tensor.matmul(ps_b, ones64, sums, start=True, stop=True)
                    sb_b = sbb.tile([D, CH], F32, tag="sb_b", name="sb_b")
                    nc.vector.reciprocal_approx_fast(sb_b, ps_b)
                    rows = slice(par * D, (par + 1) * D)
                    if par == 0:
                        nc.vector.tensor_mul(oT[rows, ot, sl], pv[0:D, :], sb_b)
                    else:
                        tmp = tmpp.tile([D, CH], BF16, tag="tmp", name="tmp")
                        nc.vector.tensor_mul(tmp, pv[0:D, :], sb_b)
                        nc.sync.dma_start(out=oT[rows, ot, sl], in_=tmp)

                def emit_vproj2(vt):
                    """Second-half V projection (heads HPC..2*HPC-1) as PE
                    filler in early pairs; kv block prefetched in phase 1."""
                    pv = psPV.tile([P, VCH], F32, tag="ps_pv", name="pv2")
                    for ct in range(CT):
                        nc.tensor.matmul(
                            pv,
                            kvb_tiles[vt][:, ct, :],
                            wv1_sb[:, ct, :],
                            start=(ct == 0),
                            stop=(ct == CT - 1),
                        )
                    nc.vector.tensor_copy(
                        v_aug[:, vt, HPC : 2 * HPC, 0:D],
                        pv.rearrange("p (h d) -> p h d", d=D),
                    )

                def emit_oproj(u, ct0, ct1, mode, pool=None):
                    """Partial O-projection over ct0..ct1-1 for unit u.
                    mode: 'init' writes y_mid, 'accum' adds to it, 'final'
                    adds the last partial and DMAs the row out.
                    The pair-7-interleaved finals pass their own pool (carved
                    from the freed scores banks) so they never clobber
                    in-flight PV tiles in the ps_pv ring."""
                    yt, vch = divmod(u, NVCH)
                    sl = slice(vch * VCH, (vch + 1) * VCH)
                    ps = (pool or psPV).tile([P, VCH], F32, tag="ps_pv", name="ps_o")
                    for ct in range(ct0, ct1):
                        nc.tensor.matmul(
                            ps,
                            oT[:, ct, yt * P : (yt + 1) * P],
                            wp_sb[:, ct, sl],
                            start=(ct == ct0),
                            stop=(ct == ct1 - 1),
                        )
                    if mode == "init":
                        nc.vector.tensor_copy(y_mid[:, yt, sl], ps)
                    elif mode == "accum":
                        nc.vector.tensor_add(y_mid[:, yt, sl], ps, y_mid[:, yt, sl])
                    else:
                        y_sb = yp.tile([P, VCH], F32, tag="y_sb", name="y_sb")
                        nc.vector.tensor_add(y_sb, ps, y_mid[:, yt, sl])
                        nc.sync.dma_start(out=y_r[:, yt, sl], in_=y_sb)

                NPAIR = H // 2
                nunits = (LQ // P) * NVCH      # 16 O-proj units per ct-range

                _psO2_box = [None]

                def get_psO2():
                    return _psO2_box[0]

                # PE filler per pair (keeps the HAM clock-gate warm while the
                # ACT engine works through the Exp stream):
                #   pair 0-1:  V-proj second half (6 + 2 lkv tiles)
                #   pairs 2-4: O-proj ct 0-2 init   (needs pairs 0-1 done)
                #   pairs 5-6: O-proj ct 2-4 accum  (needs pairs 2-3 done)
                #   pair 7:    O-proj ct 4-6 accum  (needs pairs 4-5 done)
                #   tail:      PV(pair 7) + O-proj ct 6-8 + y writeout
                filler = {pi: [] for pi in range(NPAIR)}
                for vt in range(KT):
                    filler[min(vt // 6, 1)].append(lambda vt=vt: emit_vproj2(vt))
                for u in range(nunits):
                    filler[2 + u // 6].append(
                        lambda u=u: emit_oproj(u, 0, 2, "init")
                    )
                    filler[5 + u // 8].append(
                        lambda u=u: emit_oproj(u, 2, 4, "accum")
                    )
                    filler[7].append(
                        lambda u=u: emit_oproj(u, 4, 6, "accum")
                    )

                def pv_steps_for(pair, pt01, ch_major=False, extra_by_unit=None):
                    """PV units software-pipelined: unit j's (DVE-gated) tail
                    is emitted after unit j+1's matmuls so the in-order PE
                    queue never waits on the sum-reciprocal chain.
                    extra_by_unit: {unit_idx: [callables]} appended right
                    after that unit's tail (used to interleave the final
                    O-proj units into pair 7)."""
                    if ch_major:
                        units = [(hp, ch) for ch in range(NCH)
                                 for hp in range(HPT)]
                    else:
                        units = [(hp, ch) for hp in range(HPT)
                                 for ch in range(NCH)]
                    n = len(units)
                    pvs = [None] * n
                    steps = []

                    def mk_mms(j):
                        def f():
                            hp, ch = units[j]
                            pvs[j] = pv_mms(pair, hp, ch, pt01)
                        return f

                    def mk_tail(j):
                        def f():
                            hp, ch = units[j]
                            pv_tail(pair[0] + hp, ch, pvs[j])
                        return f

                    for i in range(n + 1):
                        if i < n:
                            steps.append(mk_mms(i))
                        if 0 <= i - 1 < n:
                            steps.append(mk_tail(i - 1))
                            if extra_by_unit and (i - 1) in extra_by_unit:
                                steps.extend(extra_by_unit[i - 1])
                    return steps

                pend = None   # steps of the previous pair's PV work
                for pi in range(NPAIR):
                    pair = (2 * pi, 2 * pi + 1)
                    ot = pi
                    pt01 = ptp.tile([P, KT, HPT, LQ], BF16, tag="pt", name="pt01")
                    psteps = (pend or []) + filler[pi]
                    np_done = 0
                    for kt in range(KT):
                        emit_scores_step(ot, kt, pt01)
                        want = (kt + 1) * len(psteps) // KT
                        while np_done < want:
                            psteps[np_done]()
                            np_done += 1
                    while np_done < len(psteps):
                        psteps[np_done]()
                        np_done += 1
                    if pi < NPAIR - 1:
                        pend = pv_steps_for(pair, pt01)
                    else:
                        # Pair 7: ch-major PV units; interleave the final
                        # O-proj units as soon as their token block's oT is
                        # complete (ch0 tails done -> yt 0-3, ch1 -> yt 4-7).
                        extra = {
                            1: [lambda u=u: emit_oproj(u, 6, CT, "final",
                                                       pool=get_psO2())
                                for u in range(0, 8)],
                            3: [lambda u=u: emit_oproj(u, 6, CT, "final",
                                                       pool=get_psO2())
                                for u in range(8, nunits)],
                        }
                        pend = pv_steps_for(
                            pair, pt01, ch_major=True, extra_by_unit=extra
                        )
                # Scores are done; free the 4 psSc banks and run the tail
                # (pair 7 PV + interleaved ct 6-8 finals) with a dedicated
                # O-proj pool carved out of the freed space.
                psSc_ctx.close()
                psO2 = p2.enter_context(
                    tc.tile_pool(name="psO2", bufs=3, space="PSUM")
                )
                _psO2_box[0] = psO2
                for s in pend:
                    s()

    nc.finalize()
    return nc


_NC_CACHE = {}


def _get_nc(C, H, LQ, LKV, knobs=None):
    key = (C, H, LQ, LKV, tuple(sorted((knobs or {}).items())))
    if key not in _NC_CACHE:
        _NC_CACHE[key] = build_nc(C, H, LQ, LKV, knobs=knobs)
    return _NC_CACHE[key]


def _host_inputs(q, kv, Wq, Wkv, Wproj, bproj, tau, H):
    B, LQ, C = q.shape
    P, D = 128, C // H
    HPT = P // D

    f16 = lambda a: np.ascontiguousarray(
        np.asarray(a, dtype=np.float32).astype(np.float16)
    )
    bf16 = lambda a: np.ascontiguousarray(
        np.asarray(a, dtype=np.float32).astype(ml_dtypes.bfloat16)
    )

    wqT = f16(np.asarray(Wq).T)
    wkT = f16(np.asarray(Wkv)[:C].T)
    wvT = f16(np.asarray(Wkv)[C:].T)
    wpT = bf16(np.asarray(Wproj).T)
    tau2 = np.full((HPT, 1), float(np.asarray(tau)) ** 2, dtype=np.float32)
    ones_blk = np.zeros((P, HPT), dtype=np.float16)
    for p in range(P):
        ones_blk[p, p // D] = 1.0
    blk2 = np.ascontiguousarray(ones_blk.T)

    shared = {
        "wqT": wqT, "wkT": wkT, "wvT": wvT, "wpT": wpT,
        "tau2": tau2, "ones_blk": ones_blk, "blk2": blk2,
    }
    qn = np.asarray(q, dtype=np.float32)
    kvn = np.asarray(kv, dtype=np.float32)
    in_maps = []
    for b in range(B):
        m = dict(shared)
        m["qT"] = f16(qn[b].T)
        m["kvT"] = f16(kvn[b].T)
        in_maps.append(m)
    return in_maps


def kernel(q, kv, Wq, Wkv, Wproj, bproj, tau, _trace=False, _knobs=None):
    B, LQ, C = q.shape
    LKV = kv.shape[1]
    H = 16 if C == 1024 else max(1, C // 64)
    assert B == NCORES, f"expected B == {NCORES}, got {B}"

    nc = _get_nc(C, H, LQ, LKV, knobs=_knobs)
    in_maps = _host_inputs(q, kv, Wq, Wkv, Wproj, bproj, tau, H)
    res = run_bass_kernel_spmd(
        nc, in_maps, core_ids=list(range(NCORES)), trace=_trace
    )
    bp = np.asarray(bproj, dtype=np.float64).reshape(1, C)
    out = np.stack(
        [res.results[b]["y"].astype(np.float64) + bp for b in range(B)], axis=0
    )
    out = out.astype(np.asarray(q).dtype)
    if _trace:
        kernel._last_result = res
    return out


# revision 32
# speedup vs baseline: 1.0764x; 1.0764x over previous
"""CrossAttention (cosine-sim, learnable temperature) Trainium2 kernel, v4.

Math (per batch element b, reference in fp32):
    qh  = (q @ Wq.T)   -> [Lq, C] -> heads [H, Lq, D]
    k,v = (kv @ Wkv.T) -> k,v [H, Lkv, D]
    qn = qh / ||qh||_d; kn = k / ||k||_d
    attn = softmax(qn @ kn.T / tau); out = attn @ v
    y = out @ Wproj.T + bproj         (bproj added on host)

Distribution: pure data-parallel over B=8 across the 8 NeuronCores (one
batch element per core, weights replicated, no collectives).

v4 design notes (changes vs v2 baseline, driven by NTFF profiles):
  * Both heads of a pair share ONE 4-bank PSUM scores tile and ONE
    bf16 pt tile, so the softmax Exp runs at N=2048 instead of 2x
    N=1024 - 64 ACTIVATEs instead of 128, amortizing the ~480ns
    per-instruction overhead.  Measured: 15.7us of Exp per pair vs
    21.4us in v2.  The scores tile is single-buffered (4 banks); the
    interleaved PV/filler matmuls cover the Exp latency between kt
    steps.
  * Dummy zero-scale Sqrt/Exp activations preload the ACT table sets
    (sqrt set at kernel start, exp set during the phase boundary), so
    the first real Exp doesn't stall scores - in the v2 trace that
    stall tipped the PE into a 37us HAM re-throttle covering pairs
    0-1.
  * The norm chain and the softmax-sum normalization use PE
    ones-matmul broadcasts exactly as v2 (a v3 experiment with
    SBUF->SBUF broadcast DMAs was 2x slower: the DMA path runs at
    ~58GB/s and its queue backs up ~20us).
  * kv-ch0 input DMAs are issued before the wk columns (the first K
    job needs all kv chunks but only one wk column), cutting the DMA
    lead-in before the first matmul.
  * The phase-2 V-proj kv blocks and wv second half are DMA'd during
    phase 1 (their pools are allocated up front), so pair 0's filler
    never waits on DMA at the phase boundary.
  * O-proj filler spread over 3 partial stages (ct 0-2 at pairs 2-4,
    2-4 at 5-6, 4-6 during pair 7); the ct 6-8 finals interleave into
    pair 7's PV steps via a dedicated PSUM pool carved from the freed
    scores banks, shrinking the ACT-idle tail.
"""

import sys

sys.path.insert(0, "/opt/trn_rl_repo")

import numpy as np
import ml_dtypes

import concourse.bass as bass
import concourse.bacc as bacc
import concourse.mybir as mybir
from concourse.tile import TileContext
from concourse.bass_utils import run_bass_kernel_spmd

AF = mybir.ActivationFunctionType
F32 = mybir.dt.float32
F32R = mybir.dt.float32r
F16 = mybir.dt.float16
BF16 = mybir.dt.bfloat16

NCORES = 8


DEFAULT_KNOBS = dict(
    psA_bufs=4, psS_bufs=2, psB_bufs=2,
    sq_bufs=3, smalls_bufs=4, rbs_bufs=2,
    psSc_bufs=1, psPV_bufs=3, psBc_bufs=1,
    pt_bufs=2, rsum_bufs=2, sbb_bufs=2, tmp_bufs=2, y_bufs=2,
)


def build_nc(C=1024, H=16, LQ=1024, LKV=1024, knobs=None):
    kb = dict(DEFAULT_KNOBS)
    if knobs:
        kb.update(knobs)
    P = 128
    D = C // H            # head dim (64)
    OT = C // P           # feature tiles (8)
    CT = C // P           # contraction tiles (8)
    KT = LKV // P         # lkv partition tiles (8)
    HPT = P // D          # heads per 128-tile (2)
    CH = min(512, LQ)     # free-dim chunk per psum bank (fp32)
    NCH = LQ // CH        # chunks of Lq (2)
    VCH = min(512, C)     # chunk of output features for V projection
    NVCH = C // VCH
    HPC = VCH // D        # heads per v-projection chunk (8)

    nc = bacc.Bacc("TRN2", target_bir_lowering=False)

    qT = nc.dram_tensor("qT", [C, LQ], F16, kind="ExternalInput")
    kvT = nc.dram_tensor("kvT", [C, LKV], F16, kind="ExternalInput")
    wqT = nc.dram_tensor("wqT", [C, C], F16, kind="ExternalInput")
    wkT = nc.dram_tensor("wkT", [C, C], F16, kind="ExternalInput")
    wvT = nc.dram_tensor("wvT", [C, C], F16, kind="ExternalInput")
    wpT = nc.dram_tensor("wpT", [C, C], BF16, kind="ExternalInput")
    tau2 = nc.dram_tensor("tau2", [HPT, 1], F32, kind="ExternalInput")
    ones_blk = nc.dram_tensor("ones_blk", [P, HPT], F16, kind="ExternalInput")
    blk2 = nc.dram_tensor("blk2", [HPT, P], F16, kind="ExternalInput")
    y = nc.dram_tensor("y", [LQ, C], F32, kind="ExternalOutput")

    qT_r = qT.rearrange("(ct p) l -> p ct l", p=P)
    kvT_r = kvT.rearrange("(ct p) l -> p ct l", p=P)
    wqT_r = wqT.rearrange("(ct p) o -> p ct o", p=P)
    wkT_r = wkT.rearrange("(ct p) o -> p ct o", p=P)
    wvT_r = wvT.rearrange("(ct p) o -> p ct o", p=P)
    wpT_r = wpT.rearrange("(ct p) o -> p ct o", p=P)
    y_r = y.rearrange("(yt p) o -> p yt o", p=P)

    with TileContext(nc) as tc:
        from contextlib import ExitStack

        with ExitStack() as stk:
            # ---------- persistent pools --------------------------------
            persist = stk.enter_context(tc.tile_pool(name="persist", bufs=1))
            qnT = persist.tile([P, OT, LQ], F16)            # qh * rq
            knT = persist.tile([P, OT, LKV], F16)           # kh * rk / tau
            v_aug = persist.tile([P, KT, H, D + 1], BF16)   # [v | ones]
            oT = persist.tile([P, CT, LQ], BF16)            # (attn@v)/sum
            wp_sb = persist.tile([P, CT, C], BF16)
            consts = stk.enter_context(tc.tile_pool(name="consts", bufs=1))
            ones_blk_sb = consts.tile([P, HPT], F16)
            blk2_sb = consts.tile([HPT, P], F16)
            tau2_sb = consts.tile([HPT, 1], F32)
            ones64 = consts.tile([1, D], BF16)
            scr = consts.tile([HPT, 1], F32)

            nc.sync.dma_start(out=ones_blk_sb, in_=ones_blk[:, :])
            nc.sync.dma_start(out=blk2_sb, in_=blk2[:, :])
            nc.sync.dma_start(out=tau2_sb, in_=tau2[:, :])
            nc.vector.memset(ones64, 1.0)
            nc.vector.memset(v_aug[:, :, :, D : D + 1], 1.0)
            # Preload the sqrt ACT table set before the first real Sqrt.
            nc.scalar.activation(scr, tau2_sb, AF.Sqrt, scale=0.0)

            # kv chunks and the FULL wv outlive phase 1: the entire V
            # projection runs in phase 2 as PE filler under the Exp stream
            # (keeps the HAM clock-gate at 2.4GHz through the scores<->Exp
            # serial chain and shortens the PE-dense phase 1).
            p2v = stk.enter_context(tc.tile_pool(name="p2v", bufs=1))
            kv_tiles = [[p2v.tile([P, CH], F16, name=f"kv_{ct}_{ch}")
                         for ch in range(NCH)] for ct in range(CT)]
            wv_sb = p2v.tile([P, CT, C], F16)

            # ---------- phase 1 (scoped so pools free before phase 2) ----
            p1 = ExitStack()
            # kv ch0 chunks first (the first K job needs all of them but
            # only one wk column), then wk/wv columns, kv ch1, then q + q
            # weights (phase 1b), O-proj weights and phase-2 V inputs last.
            # Inputs are split into per-chunk TILES (not slices of one big
            # tile): the Tile dependency tracker is tile-granular across
            # mismatched AP patterns, so one big tile makes the first matmul
            # wait for ALL input DMAs (~19us) instead of just the chunks it
            # reads (~8us).
            p1w = p1.enter_context(tc.tile_pool(name="p1w", bufs=1))
            wk_tiles = [p1w.tile([P, CT, P], F16, name=f"wk_{ot}")
                        for ot in range(OT)]
            q_tiles = [[p1w.tile([P, CH], F16, name=f"q_{ct}_{ch}")
                        for ch in range(NCH)] for ct in range(CT)]
            wq_tiles = [p1w.tile([P, CT, P], F16, name=f"wq_{ot}")
                        for ot in range(OT)]
            for ct in range(CT):
                nc.sync.dma_start(out=kv_tiles[ct][0], in_=kvT_r[:, ct, 0:CH])
            for ot in range(3):
                sl = slice(ot * P, (ot + 1) * P)
                nc.sync.dma_start(out=wk_tiles[ot], in_=wkT_r[:, :, sl])
            for ct in range(CT):
                nc.sync.dma_start(out=kv_tiles[ct][1], in_=kvT_r[:, ct, CH:LKV])
            for ot in range(3, OT):
                sl = slice(ot * P, (ot + 1) * P)
                nc.sync.dma_start(out=wk_tiles[ot], in_=wkT_r[:, :, sl])
            for ct in range(CT):
                nc.sync.dma_start(out=q_tiles[ct][0], in_=qT_r[:, ct, 0:CH])
                nc.sync.dma_start(out=q_tiles[ct][1], in_=qT_r[:, ct, CH:LQ])
            for ot in range(OT):
                sl = slice(ot * P, (ot + 1) * P)
                nc.sync.dma_start(out=wq_tiles[ot], in_=wqT_r[:, :, sl])
            for ct in range(CT):
                sl = slice(ct * P, (ct + 1) * P)
                nc.sync.dma_start(out=wv_sb[:, :, sl], in_=wvT_r[:, :, sl])
            for ct in range(CT):
                nc.sync.dma_start(out=wp_sb[:, ct, :], in_=wpT_r[:, ct, :])

            # ============ PHASE 1a: K norm-proj + V proj ================
            class Job:
                def A(self):
                    pass

                def B(self):
                    pass

                def Cs(self):
                    pass

            def run_pipeline(jobs):
                n = len(jobs)
                for i in range(n + 2):
                    if i < n:
                        jobs[i].A()
                    if 0 <= i - 1 < n:
                        jobs[i - 1].B()
                    if 0 <= i - 2 < n:
                        jobs[i - 2].Cs()

            with ExitStack() as p1c:
                sqp = p1c.enter_context(tc.tile_pool(name="sqp", bufs=kb["sq_bufs"]))
                smalls = p1c.enter_context(
                    tc.tile_pool(name="smalls", bufs=kb["smalls_bufs"])
                )
                rbs = p1c.enter_context(tc.tile_pool(name="rbsa", bufs=kb["rbs_bufs"]))
                psA = p1c.enter_context(
                    tc.tile_pool(name="psA", bufs=kb["psA_bufs"], space="PSUM")
                )
                psS = p1c.enter_context(
                    tc.tile_pool(name="psS", bufs=kb["psS_bufs"], space="PSUM")
                )
                psB = p1c.enter_context(
                    tc.tile_pool(name="psB", bufs=kb["psB_bufs"], space="PSUM")
                )

                class NormJob(Job):
                    """Shared K/Q norm-projection job body (v2 chain)."""

                    def __init__(self, ot, ch):
                        self.ot, self.ch = ot, ch
                        self.sl = slice(ch * CH, (ch + 1) * CH)

                    def A(self):
                        self.ph = psA.tile([P, CH], F32, tag="ph", name="ph")
                        w_tiles, x_tiles = self.srcs()
                        wcol = w_tiles[self.ot]
                        for ct in range(CT):
                            nc.tensor.matmul(
                                self.ph,
                                wcol[:, ct, :],
                                x_tiles[ct][self.ch],
                                start=(ct == 0),
                                stop=(ct == CT - 1),
                            )
                        self.sq = sqp.tile([P, CH], F16, tag="sq", name="sq")
                        nc.scalar.activation(self.sq, self.ph, AF.Square)

                    def B(self):
                        ssq = psS.tile([HPT, CH], F32, tag="ssq", name="ssq")
                        nc.tensor.matmul(ssq, ones_blk_sb, self.sq, start=True, stop=True)
                        # rr = sqrt(ssq * scale) in f16, so the broadcast
                        # matmul below runs at full f16 rate
                        self.rr = smalls.tile([HPT, CH], F16, tag="rr", name="rr")
                        nc.scalar.activation(
                            self.rr, ssq, AF.Sqrt, scale=self.sqrt_scale()
                        )

                    def Cs(self):
                        rb = psB.tile([P, CH], F32, tag="rb", name="rb")
                        nc.tensor.matmul(rb, blk2_sb, self.rr, start=True, stop=True)
                        rb_sb = rbs.tile([P, CH], F32, tag="rb_sb", name="rb_sb")
                        nc.vector.reciprocal_approx_fast(rb_sb, rb)
                        nc.vector.tensor_mul(
                            self.dst()[:, self.ot, self.sl], self.ph, rb_sb
                        )

                class KJob(NormJob):
                    def srcs(self):
                        return wk_tiles, kv_tiles

                    def sqrt_scale(self):
                        return tau2_sb

                    def dst(self):
                        return knT

                class QJob(NormJob):
                    def srcs(self):
                        return wq_tiles, q_tiles

                    def sqrt_scale(self):
                        return 1.0

                    def dst(self):
                        return qnT

                # One merged pipeline: ch0 K jobs first (they only need the
                # ch0 kv halves), ch1 K jobs as those land, then all Q jobs.
                # A single pool scope means no pipeline drain at the K/Q
                # boundary.  (V projection happens in phase 2 as filler.)
                jobs = [KJob(ot, 0) for ot in range(OT)]
                jobs += [KJob(ot, 1) for ot in range(OT)]
                jobs += [QJob(i // 2, i % 2) for i in range(2 * OT)]
                run_pipeline(jobs)

            # Preload the ACT exp table set during the phase boundary so the
            # first real Exp doesn't stall the (single-buffered) scores tile.
            nc.scalar.activation(scr, tau2_sb, AF.Exp, scale=0.0)

            # free phase-1 inputs/weights before the big pt pool allocates
            p1.close()

            # ============ PHASE 2: attention (head pairs) ===============
            with ExitStack() as p2:
                ymp = p2.enter_context(tc.tile_pool(name="ymp", bufs=1))
                y_mid = ymp.tile([P, LQ // P, C], BF16)
                ptp = p2.enter_context(tc.tile_pool(name="ptp", bufs=kb["pt_bufs"]))
                rsp = p2.enter_context(tc.tile_pool(name="rsp", bufs=kb["rsum_bufs"]))
                sbb = p2.enter_context(tc.tile_pool(name="sbb", bufs=kb["sbb_bufs"]))
                tmpp = p2.enter_context(tc.tile_pool(name="tmpp", bufs=kb["tmp_bufs"]))
                yp = p2.enter_context(tc.tile_pool(name="yp", bufs=kb["y_bufs"]))
                psPV = p2.enter_context(
                    tc.tile_pool(name="psPV", bufs=kb["psPV_bufs"], space="PSUM")
                )
                psBc = p2.enter_context(
                    tc.tile_pool(name="psBc", bufs=kb["psBc_bufs"], space="PSUM")
                )
                # psSc entered last so it can be released (LIFO) before the
                # tail, freeing its 4 banks for the psO2 pool.
                psSc_ctx = ExitStack()
                psSc = psSc_ctx.enter_context(
                    tc.tile_pool(name="psSc", bufs=kb["psSc_bufs"], space="PSUM")
                )

                def emit_scores_step(ot, kt, pt01):
                    """One kt slice of a head pair's scores + exp: both heads
                    share a 4-bank PSUM tile so the Exp runs at N=2048.
                    Matmuls are ch-major so the hp0/hp1 matmuls sit on PE
                    row groups 0-1 / 2-3 and execute concurrently - the
                    scores leg of the serial scores->Exp chain is ~2 matmul
                    times, not 4."""
                    kl = slice(kt * P, (kt + 1) * P)
                    s01 = psSc.tile([P, HPT, LQ], F32, tag="ps_s", name="s01")
                    for ch in range(NCH):
                        sl = slice(ch * CH, (ch + 1) * CH)
                        for hp in range(HPT):
                            r = slice(hp * D, (hp + 1) * D)
                            nc.tensor.matmul(
                                s01[:, hp, sl], knT[r, ot, kl], qnT[r, ot, sl],
                                start=True, stop=True,
                            )
                    nc.scalar.activation(pt01[:, kt, :, :], s01, AF.Exp)

                def pv_mms(pair, hp, ch, pt01):
                    """attn@v (+softmax sum via the ones column) matmuls for
                    one (head, Lq-chunk)."""
                    sl = slice(ch * CH, (ch + 1) * CH)
                    pv = psPV.tile([D + 1, CH], F32, tag="ps_pv", name="ps_pv")
                    for kt in range(KT):
                        nc.tensor.matmul(
                            pv,
                            v_aug[:, kt, pair[0] + hp, :],
                            pt01[:, kt, hp, sl],
                            start=(kt == 0),
                            stop=(kt == KT - 1),
                        )
                    return pv

                def pv_tail(h, ch, pv):
                    """Softmax-sum fast-recip after a PE ones-broadcast,
                    then normalize into oT (v2 chain)."""
                    par, ot = h % HPT, h // HPT
                    sl = slice(ch * CH, (ch + 1) * CH)
                    sums = rsp.tile([1, CH], BF16, tag="rsum", name="sums")
                    nc.vector.tensor_copy(sums, pv[D : D + 1, :])
                    ps_b = psBc.tile([D, CH], F32, tag="ps_b", name="ps_b")
                    nc.tensor.matmul(ps_b, ones64, sums, start=True, stop=True)
                    sb_b = sbb.tile([D, CH], F32, tag="sb_b", name="sb_b")
                    nc.vector.reciprocal_approx_fast(sb_b, ps_b)
                    rows = slice(par * D, (par + 1) * D)
                    if par == 0:
                        nc.vector.tensor_mul(oT[rows, ot, sl], pv[0:D, :], sb_b)
                    else:
                        tmp = tmpp.tile([D, CH], BF16, tag="tmp", name="tmp")
                        nc.vector.tensor_mul(tmp, pv[0:D, :], sb_b)
                        nc.sync.dma_start(out=oT[rows, ot, sl], in_=tmp)

                def emit_vproj(vch, vt):
                    """One V-projection unit (8 heads x one lkv tile) as PE
                    filler; inputs (kv chunks + full wv) persist from
                    phase 1."""
                    pv = psPV.tile([P, VCH], F32, tag="ps_pv", name="pv2")
                    wcol = wv_sb[:, :, vch * VCH : (vch + 1) * VCH]
                    vch_blk, off = divmod(vt * P, CH)
                    for ct in range(CT):
                        nc.tensor.matmul(
                            pv,
                            kv_tiles[ct][vch_blk][:, off : off + P],
                            wcol[:, ct, :],
                            start=(ct == 0),
                            stop=(ct == CT - 1),
                        )
                    nc.vector.tensor_copy(
                        v_aug[:, vt, vch * HPC : (vch + 1) * HPC, 0:D],
                        pv.rearrange("p (h d) -> p h d", d=D),
                    )

                def emit_oproj(u, ct0, ct1, mode, pool=None):
                    """Partial O-projection over ct0..ct1-1 for unit u.
                    mode: 'init' writes y_mid, 'accum' adds to it, 'final'
                    adds the last partial and DMAs the row out.
                    The pair-7-interleaved finals pass their own pool (carved
                    from the freed scores banks) so they never clobber
                    in-flight PV tiles in the ps_pv ring."""
                    yt, vch = divmod(u, NVCH)
                    sl = slice(vch * VCH, (vch + 1) * VCH)
                    ps = (pool or psPV).tile([P, VCH], F32, tag="ps_pv", name="ps_o")
                    for ct in range(ct0, ct1):
                        nc.tensor.matmul(
                            ps,
                            oT[:, ct, yt * P : (yt + 1) * P],
                            wp_sb[:, ct, sl],
                            start=(ct == ct0),
                            stop=(ct == ct1 - 1),
                        )
                    if mode == "init":
                        nc.vector.tensor_copy(y_mid[:, yt, sl], ps)
                    elif mode == "accum":
                        nc.vector.tensor_add(y_mid[:, yt, sl], ps, y_mid[:, yt, sl])
                    else:
                        y_sb = yp.tile([P, VCH], F32, tag="y_sb", name="y_sb")
                        nc.vector.tensor_add(y_sb, ps, y_mid[:, yt, sl])
                        nc.sync.dma_start(out=y_r[:, yt, sl], in_=y_sb)

                NPAIR = H // 2
                nunits = (LQ // P) * NVCH      # 16 O-proj units per ct-range

                _psO2_box = [None]

                def get_psO2():
                    return _psO2_box[0]

                # PE filler per pair (keeps the HAM clock-gate warm while the
                # ACT engine works through the Exp stream):
                #   pairs 0-1: V-proj heads 0-7 (needed by pair-0 PV, which
                #              runs during pair 1)
                #   pairs 1-3: V-proj heads 8-15 (needed by pair-4 PV)
                #   pairs 2-4: O-proj ct 0-2 init   (needs pairs 0-1 done)
                #   pairs 5-6: O-proj ct 2-4 accum  (needs pairs 2-3 done)
                #   pair 7:    O-proj ct 4-6 accum  (needs pairs 4-5 done)
                #   tail:      PV(pair 7) + O-proj ct 6-8 + y writeout
                filler = {pi: [] for pi in range(NPAIR)}
                for vt in range(KT):
                    # vch0 (heads 0-7): pair-0 PV (heads 0-1) runs as pend
                    # during pair 1 and reads all kt, so every vch0 unit must
                    # be emitted within pair 0.  vch1 (heads 8-15): first
                    # read by pair-4 PV during pair 5; spread over pairs 1-4.
                    filler[0].append(lambda vt=vt: emit_vproj(0, vt))
                    filler[1 + vt // 2].append(lambda vt=vt: emit_vproj(1, vt))
                for u in range(nunits):
                    filler[2 + u // 6].append(
                        lambda u=u: emit_oproj(u, 0, 2, "init")
                    )
                    filler[5 + u // 8].append(
                        lambda u=u: emit_oproj(u, 2, 4, "accum")
                    )
                    filler[7].append(
                        lambda u=u: emit_oproj(u, 4, 6, "accum")
                    )

                def pv_steps_for(pair, pt01, ch_major=False, extra_by_unit=None):
                    """PV units software-pipelined: unit j's (DVE-gated) tail
                    is emitted after unit j+1's matmuls so the in-order PE
                    queue never waits on the sum-reciprocal chain.
                    extra_by_unit: {unit_idx: [callables]} appended right
                    after that unit's tail (used to interleave the final
                    O-proj units into pair 7)."""
                    if ch_major:
                        units = [(hp, ch) for ch in range(NCH)
                                 for hp in range(HPT)]
                    else:
                        units = [(hp, ch) for hp in range(HPT)
                                 for ch in range(NCH)]
                    n = len(units)
                    pvs = [None] * n
                    steps = []

                    def mk_mms(j):
                        def f():
                            hp, ch = units[j]
                            pvs[j] = pv_mms(pair, hp, ch, pt01)
                        return f

                    def mk_tail(j):
                        def f():
                            hp, ch = units[j]
                            pv_tail(pair[0] + hp, ch, pvs[j])
                        return f

                    for i in range(n + 1):
                        if i < n:
                            steps.append(mk_mms(i))
                        if 0 <= i - 1 < n:
                            steps.append(mk_tail(i - 1))
                            if extra_by_unit and (i - 1) in extra_by_unit:
                                steps.extend(extra_by_unit[i - 1])
                    return steps

                pend = None   # steps of the previous pair's PV work
                for pi in range(NPAIR):
                    pair = (2 * pi, 2 * pi + 1)
                    ot = pi
                    pt01 = ptp.tile([P, KT, HPT, LQ], BF16, tag="pt", name="pt01")
                    psteps = (pend or []) + filler[pi]
                    np_done = 0
                    for kt in range(KT):
                        emit_scores_step(ot, kt, pt01)
                        want = (kt + 1) * len(psteps) // KT
                        while np_done < want:
                            psteps[np_done]()
                            np_done += 1
                    while np_done < len(psteps):
                        psteps[np_done]()
                        np_done += 1
                    if pi < NPAIR - 1:
                        pend = pv_steps_for(pair, pt01)
                    else:
                        # Pair 7: ch-major PV units; interleave the final
                        # O-proj units as soon as their token block's oT is
                        # complete (ch0 tails done -> yt 0-3, ch1 -> yt 4-7).
                        extra = {
                            1: [lambda u=u: emit_oproj(u, 6, CT, "final",
                                                       pool=get_psO2())
                                for u in range(0, 8)],
                            3: [lambda u=u: emit_oproj(u, 6, CT, "final",
                                                       pool=get_psO2())
                                for u in range(8, nunits)],
                        }
                        pend = pv_steps_for(
                            pair, pt01, ch_major=True, extra_by_unit=extra
                        )
                # Scores are done; free the 4 psSc banks and run the tail
                # (pair 7 PV + interleaved ct 6-8 finals) with a dedicated
                # O-proj pool carved out of the freed space.
                psSc_ctx.close()
                psO2 = p2.enter_context(
                    tc.tile_pool(name="psO2", bufs=3, space="PSUM")
                )
                _psO2_box[0] = psO2
                for s in pend:
                    s()

    nc.finalize()
    return nc


_NC_CACHE = {}


def _get_nc(C, H, LQ, LKV, knobs=None):
    key = (C, H, LQ, LKV, tuple(sorted((knobs or {}).items())))
    if key not in _NC_CACHE:
        _NC_CACHE[key] = build_nc(C, H, LQ, LKV, knobs=knobs)
    return _NC_CACHE[key]


def _host_inputs(q, kv, Wq, Wkv, Wproj, bproj, tau, H):
    B, LQ, C = q.shape
    P, D = 128, C // H
    HPT = P // D

    f16 = lambda a: np.ascontiguousarray(
        np.asarray(a, dtype=np.float32).astype(np.float16)
    )
    bf16 = lambda a: np.ascontiguousarray(
        np.asarray(a, dtype=np.float32).astype(ml_dtypes.bfloat16)
    )

    wqT = f16(np.asarray(Wq).T)
    wkT = f16(np.asarray(Wkv)[:C].T)
    wvT = f16(np.asarray(Wkv)[C:].T)
    wpT = bf16(np.asarray(Wproj).T)
    tau2 = np.full((HPT, 1), float(np.asarray(tau)) ** 2, dtype=np.float32)
    ones_blk = np.zeros((P, HPT), dtype=np.float16)
    for p in range(P):
        ones_blk[p, p // D] = 1.0
    blk2 = np.ascontiguousarray(ones_blk.T)

    shared = {
        "wqT": wqT, "wkT": wkT, "wvT": wvT, "wpT": wpT,
        "tau2": tau2, "ones_blk": ones_blk, "blk2": blk2,
    }
    qn = np.asarray(q, dtype=np.float32)
    kvn = np.asarray(kv, dtype=np.float32)
    in_maps = []
    for b in range(B):
        m = dict(shared)
        m["qT"] = f16(qn[b].T)
        m["kvT"] = f16(kvn[b].T)
        in_maps.append(m)
    return in_maps


def kernel(q, kv, Wq, Wkv, Wproj, bproj, tau, _trace=False, _knobs=None):
    B, LQ, C = q.shape
    LKV = kv.shape[1]
    H = 16 if C == 1024 else max(1, C // 64)
    assert B == NCORES, f"expected B == {NCORES}, got {B}"

    nc = _get_nc(C, H, LQ, LKV, knobs=_knobs)
    in_maps = _host_inputs(q, kv, Wq, Wkv, Wproj, bproj, tau, H)
    res = run_bass_kernel_spmd(
        nc, in_maps, core_ids=list(range(NCORES)), trace=_trace
    )
    bp = np.asarray(bproj, dtype=np.float64).reshape(1, C)
    out = np.stack(
        [res.results[b]["y"].astype(np.float64) + bp for b in range(B)], axis=0
    )
    out = out.astype(np.asarray(q).dtype)
    if _trace:
        kernel._last_result = res
    return out
